# revision 15
# baseline (speedup 1.0000x reference)
"""Trainium2 Bass kernel for MetaLayer-style GNN (edge/node/global GRU message passing).

Contract: kernel(**inputs) takes the FULL unsharded inputs (np arrays, keys as in
setup_inputs) and returns the FULL output [B, STEPS, H] float32.

Strategy (8 NeuronCores), v4:
- Sort edges by dst, shard nodes into 8 equal contiguous ranges; each core owns all
  edges whose dst is in its range => node aggregation is core-local.
- Each shard is split into a lo-section (src < 32768) and a hi-section, each
  dst-sorted, so x[src] is fetched with single-row 256B dma_gathers from two
  base offsets of the replicated x table (int16 index limit) - no pair
  gather, no parity merge.
- Per-chunk streams packed into ONE DRAM "arena" row per chunk
  (eT | Dmat(fp8) | Amat(bf16)) + a small S_u row: 3 HWDGE dispatches/chunk.
- x[dst] expansion via pre-projected PX rows at two 64-alignments (fp8) and
  fp8 one-hot Dmat matmuls.
- GRU: h-side matmuls emitted before the previous chunk's transpose+aggregate
  block, x-side after, so the PE queue never stalls on Scalar/Vector and HAM
  stays warm. Zero-bias fast path skips the hg bias-add.
- Edge state eT round-trips DRAM through the arena (ping-pong parity).
- AllGather of updated x (bf16) right after the node phase; small AllReduce
  for per-graph node means.
"""

from contextlib import ExitStack

import numpy as np
import ml_dtypes

import concourse.bass as bass
import concourse.bacc as bacc
import concourse.tile as tile
from concourse import mybir
from concourse.bass_utils import run_bass_kernel_spmd
from concourse.masks import make_identity

BF16 = ml_dtypes.bfloat16
FP8 = ml_dtypes.float8_e4m3
AF = mybir.ActivationFunctionType
DT = mybir.dt
ALU = mybir.AluOpType

SPLIT = 32768                             # lo/hi src split (int16 idx limit)

# ---------------------------------------------------------------- configuration


class Cfg:
    def __init__(self, N=50000, E=500000, B=64, H=128, STEPS=3, NCORES=8,
                 CH=512, GB=2048):
        assert H == 128
        assert N % NCORES == 0
        self.N, self.E, self.B, self.H, self.STEPS, self.NCORES = N, E, B, H, STEPS, NCORES
        self.CH = CH                      # edge chunk (free dim of f32 PSUM <= 512)
        self.GB = GB                      # edges per dma_gather call
        self.CPB = GB // CH               # chunks per gather batch
        self.NL = N // NCORES             # local nodes
        self.NLP = ((self.NL + CH - 1) // CH) * CH
        self.NCHN = self.NLP // CH        # node chunks
        self.NBLK = self.NLP // 128       # 128-node blocks
        self.AW = 256                     # aggregation window width (nodes)

        # arena byte layout per (chunk, partition)
        self.ET_OFF = 0                   # 512 bf16 = 1024B
        self.D0_OFF = 1024                # 512 fp8
        self.D1_OFF = 1536                # 512 fp8
        self.A_OFF = 2048                 # 4 x 256 bf16 = 4 x 512B
        self.AR_W = 4096

        self.zero_gbias = False           # set by host_prepare

    def finalize(self, max_lo, max_hi):
        gb = self.GB
        self.EPAD_LO = ((max_lo + gb - 1) // gb) * gb
        self.EPAD_HI = ((max_hi + gb - 1) // gb) * gb
        self.EPAD = self.EPAD_LO + self.EPAD_HI
        self.NBAT_LO = self.EPAD_LO // gb
        self.NCHE_LO = self.EPAD_LO // self.CH
        self.NSUBS_LO = self.EPAD_LO // 128
        self.NCHE = self.EPAD // self.CH
        self.NSUBS = self.EPAD // 128
        # aggregation-window group id per sub: quantile-derived, breaks at the
        # lo/hi boundary (window STARTS are set data-dependently in
        # set_windows from the union of all cores' spans).
        self.grp = []
        g, prev_q = -1, None
        for gs in range(self.NSUBS):
            if gs < self.NSUBS_LO:
                c = (gs + 0.5) * 128 * self.NL / self.EPAD_LO
            else:
                c = (gs - self.NSUBS_LO + 0.5) * 128 * self.NL / self.EPAD_HI
            q = int(c // 128)
            if gs == 0 or gs == self.NSUBS_LO or q != prev_q:
                g += 1
            prev_q = q
            self.grp.append(g)
        self.NGRP = self.grp[-1] + 1
        return self

    def set_windows(self, grp_min, grp_max, ch_min, ch_max):
        """Window starts from the union (over cores) of real-edge dst spans.
        grp_*: [NGRP] per aggregation group; ch_*: [NCHE] per chunk (64-grid)."""
        self.wgrp = []
        for g in range(self.NGRP):
            w = 0 if grp_min[g] > grp_max[g] else int(grp_min[g])
            w = max(0, min(w, self.NLP - self.AW))
            assert grp_max[g] < w + self.AW, \
                f"agg window violated: grp {g} [{grp_min[g]},{grp_max[g]}] w={w}"
            self.wgrp.append(w)
        self.wstart = [self.wgrp[self.grp[gs]] for gs in range(self.NSUBS)]
        self.w2start = []
        for k in range(self.NCHE):
            w = 0 if ch_min[k] > ch_max[k] else 64 * (int(ch_min[k]) // 64)
            w = max(0, min(w, self.NLP - self.AW))
            assert ch_max[k] < w + self.AW, \
                f"dst window violated: chunk {k} [{ch_min[k]},{ch_max[k]}] w={w}"
            self.w2start.append(w)


# ---------------------------------------------------------------- host helpers


def _wrap16(idx, call):
    """Pack indices into the wrapped-16, replicated-128 layout of dma_gather:
    element [p, c*(call//16) + s] = idx[c*call + s*16 + p%16]."""
    total = idx.shape[0]
    assert total % call == 0 and call % 16 == 0
    w = idx.reshape(total // call, call // 16, 16)            # [c, s, lane]
    w = np.transpose(w, (2, 0, 1)).reshape(16, total // 16)   # [lane, c*s]
    w = np.tile(w, (8, 1))                                    # -> 128 partitions
    return np.ascontiguousarray(w.astype(np.int16))


def _onehot(cols_idx, nrows, scale=None, dtype=BF16):
    """[nrows, len(cols_idx)]: out[cols_idx[j], j] = scale_j; idx<0 -> zero col."""
    ncols = cols_idx.shape[0]
    out = np.zeros((nrows, ncols), dtype=np.float32)
    j = np.nonzero(cols_idx >= 0)[0]
    s = np.ones(j.shape[0], np.float32) if scale is None else scale[j]
    out[cols_idx[j], j] = s
    return out.astype(dtype)


def _quantile_slots(dst_sec, epad_h, NL):
    """Slot for each dst-sorted edge: tracks the dst-proportional line
    (slot ~ dst * epad_h/NL) so every core's chunk k covers the same node
    window; local degree pileups push edges at most a few slots past the
    line (pad slack pulls them back), the tail clamp keeps everything in
    range. Strictly increasing."""
    ne = dst_sec.shape[0]
    if ne == 0:
        return np.empty(0, np.int64)
    scale = epad_h / NL
    p = np.floor(dst_sec * scale).astype(np.int64)
    i = np.arange(ne)
    s = i + np.maximum.accumulate(p - i)
    return np.minimum(s, epad_h - ne + i)


def host_prepare(cfg, inputs):
    N, E, B, H = cfg.N, cfg.E, cfg.B, cfg.H
    x = np.asarray(inputs['x'], np.float32)
    edge_index = np.asarray(inputs['edge_index'])
    edge_attr = np.asarray(inputs['edge_attr'], np.float32)
    u = np.asarray(inputs['u'], np.float32)
    batch = np.asarray(inputs['batch']).astype(np.int64)
    src, dst = edge_index[0].astype(np.int64), edge_index[1].astype(np.int64)

    def g(name):
        return np.asarray(inputs[name], np.float32)

    W1, b1 = g('edge_w1'), g('edge_b1')
    W2, b2 = g('edge_w2'), g('edge_b2')
    eWih, eWhh = g('egru_wih'), g('egru_whh')
    eBih, eBhh = g('egru_bih'), g('egru_bhh')
    nW1, nb1 = g('node_w1'), g('node_b1')
    nW2, nb2 = g('node_w2'), g('node_b2')
    nWih, nWhh = g('ngru_wih'), g('ngru_whh')
    nBih, nBhh = g('ngru_bih'), g('ngru_bhh')
    gW1, gb1 = g('glob_w1'), g('glob_b1')
    gW2, gb2 = g('glob_w2'), g('glob_b2')
    gWih, gWhh = g('ggru_wih'), g('ggru_whh')
    gBih, gBhh = g('ggru_bih'), g('ggru_bhh')

    eWih2, eBih2 = eWih @ W2, eWih @ b2 + eBih
    nWih2, nBih2 = nWih @ nW2, nWih @ nb2 + nBih
    gWih2, gBih2 = gWih @ gW2, gWih @ gb2 + gBih

    def gate(Wm, i):
        return Wm[i * H:(i + 1) * H, :].T

    blocks = [
        W1[:, 0:H].T, W1[:, H:2 * H].T, W1[:, 2 * H:3 * H].T, W1[:, 3 * H:4 * H].T,
        gate(eWih2, 0), gate(eWih2, 1), gate(eWih2, 2),
        gate(eWhh, 0), gate(eWhh, 1), gate(eWhh, 2),
        nW1[:, 0:H].T, nW1[:, H:2 * H].T, nW1[:, 2 * H:3 * H].T,
        gate(nWih2, 0), gate(nWih2, 1), gate(nWih2, 2),
        gate(nWhh, 0), gate(nWhh, 1), gate(nWhh, 2),
        gW1[:, 0:H].T, gW1[:, H:2 * H].T,
        gate(gWih2, 0), gate(gWih2, 1), gate(gWih2, 2),
        gate(gWhh, 0), gate(gWhh, 1), gate(gWhh, 2),
    ]
    wpk = np.concatenate([bl.astype(np.float32) for bl in blocks], axis=1).astype(BF16)

    def gb_(v, i):
        return v[i * H:(i + 1) * H]

    bcols = [
        b1, gb_(eBih2, 0) + gb_(eBhh, 0), gb_(eBih2, 1) + gb_(eBhh, 1), gb_(eBhh, 2), gb_(eBih2, 2),
        nb1, gb_(nBih2, 0) + gb_(nBhh, 0), gb_(nBih2, 1) + gb_(nBhh, 1), gb_(nBhh, 2), gb_(nBih2, 2),
        gb1, gb_(gBih2, 0) + gb_(gBhh, 0), gb_(gBih2, 1) + gb_(gBhh, 1), gb_(gBhh, 2), gb_(gBih2, 2),
    ]
    bpk = np.stack(bcols, axis=1).astype(np.float32)

    # zero-bias fast path: skip the hg bias-add when every hhg bias is 0
    cfg.zero_gbias = bool(
        np.all(eBhh[2 * H:] == 0) and np.all(nBhh[2 * H:] == 0)
        and np.all(gBhh[2 * H:] == 0))

    order = np.argsort(dst, kind='stable')
    ssrc, sdst, sea = src[order], dst[order], edge_attr[order]
    shard_of = sdst // cfg.NL
    is_hi_all = ssrc >= SPLIT
    max_lo = max_hi = 0
    for c in range(cfg.NCORES):
        m = shard_of == c
        nhi = int(np.count_nonzero(is_hi_all & m))
        nlo = int(np.count_nonzero(m)) - nhi
        max_lo, max_hi = max(max_lo, nlo), max(max_hi, nhi)
    cfg.finalize(max_lo, max_hi)

    gcnt = np.bincount(batch, minlength=B).astype(np.float32)
    ginv = 1.0 / np.maximum(gcnt, 1.0)
    ncnt = np.bincount(sdst, minlength=N).astype(np.float32)
    ninv = 1.0 / np.maximum(ncnt, 1.0)
    bsrc_all = batch[ssrc]

    xb = x.astype(BF16)
    in_maps = []
    bounds = np.searchsorted(sdst, np.arange(cfg.NCORES + 1) * cfg.NL)

    # ---- pass 1: slot assignment per core + union dst spans for the windows
    cores = []
    grp_arr = np.asarray(cfg.grp)
    grp_min = np.full(cfg.NGRP, 1 << 30, np.int64)
    grp_max = np.full(cfg.NGRP, -1, np.int64)
    ch_min = np.full(cfg.NCHE, 1 << 30, np.int64)
    ch_max = np.full(cfg.NCHE, -1, np.int64)
    for c in range(cfg.NCORES):
        lo_, hi_ = int(bounds[c]), int(bounds[c + 1])
        base = c * cfg.NL
        csrc_all = ssrc[lo_:hi_]
        cdst_all = sdst[lo_:hi_] - base
        hi_sel = csrc_all >= SPLIT

        # per-section slot arrays, concatenated lo|hi
        gidx = np.zeros(cfg.EPAD, np.int64)           # gather idx (junk at pads)
        sdst_loc = np.zeros(cfg.EPAD, np.int64)
        sbat = np.full(cfg.EPAD, -1, np.int64)        # batch[src] (-1 at pads)
        seaT = np.zeros((128, cfg.EPAD), BF16)
        is_pad = np.zeros(cfg.EPAD, bool)
        cea_all = sea[lo_:hi_]
        cb_all = bsrc_all[lo_:hi_]
        is_pad[:] = True
        for h, (sel, off, epad_h) in enumerate((
                (~hi_sel, 0, cfg.EPAD_LO),
                (hi_sel, cfg.EPAD_LO, cfg.EPAD_HI))):
            es = np.nonzero(sel)[0]
            slot = off + _quantile_slots(cdst_all[es], epad_h, cfg.NL)
            is_pad[slot] = False
            gidx[slot] = csrc_all[es] - (SPLIT if h else 0)
            sdst_loc[slot] = cdst_all[es]
            sbat[slot] = cb_all[es]
            seaT[:, slot] = cea_all[es].T.astype(BF16)

        eslot = np.nonzero(~is_pad)[0]
        d_e = sdst_loc[eslot]
        np.minimum.at(grp_min, grp_arr[eslot // 128], d_e)
        np.maximum.at(grp_max, grp_arr[eslot // 128], d_e)
        np.minimum.at(ch_min, eslot // cfg.CH, d_e)
        np.maximum.at(ch_max, eslot // cfg.CH, d_e)
        cores.append(dict(base=base, gidx=gidx, sdst_loc=sdst_loc, sbat=sbat,
                          seaT=seaT, eslot=eslot))
    cfg.set_windows(grp_min, grp_max, ch_min, ch_max)

    # ---- pass 2: build per-core device inputs with the union windows
    for c in range(cfg.NCORES):
        st = cores[c]
        base, gidx, sdst_loc = st['base'], st['gidx'], st['sdst_loc']
        sbat, seaT, eslot = st['sbat'], st['seaT'], st['eslot']
        nl, nlp = cfg.NL, cfg.NLP

        # Dmat: per chunk, fp8 one-hot [2, 128, CH] mapping window nodes ->
        # edge columns (x[dst] = PXrow_window contraction).
        w2 = np.asarray(cfg.w2start)                  # [NCHE]
        rel2 = sdst_loc[eslot] - w2[eslot // cfg.CH]
        assert rel2.min() >= 0 and rel2.max() < cfg.AW, \
            f"dst window violated: {rel2.min()} {rel2.max()}"
        Dmat = np.zeros((cfg.NCHE, 2, 128, cfg.CH), np.float32)
        Dmat[eslot // cfg.CH, rel2 // 128, rel2 % 128, eslot % cfg.CH] = 1.0
        Dmat = Dmat.astype(FP8)

        # A tiles: per 128-edge sub, one-hot [128, AW] with 1/cnt folded.
        ws = np.asarray(cfg.wstart)                   # [NSUBS]
        rel = sdst_loc[eslot] - ws[eslot // 128]
        assert rel.min() >= 0 and rel.max() < cfg.AW, \
            f"agg window violated: {rel.min()} {rel.max()}"
        ninv_loc = ninv[base:base + nl]
        Amat = np.zeros((cfg.NSUBS, 128, cfg.AW), np.float32)
        Amat[eslot // 128, eslot % 128, rel] = ninv_loc[sdst_loc[eslot]]
        Amat = Amat.astype(BF16)

        # arena: [2][NCHE, 128, AR_W] u8 (eT | D0 | D1 | Amat), ping-pong parity
        arena = np.zeros((2, cfg.NCHE, 128, cfg.AR_W), np.uint8)
        eT0_b = seaT.view(np.uint8).reshape(128, cfg.NCHE, 2 * cfg.CH)
        arena[0, :, :, cfg.ET_OFF:cfg.ET_OFF + 2 * cfg.CH] = \
            np.transpose(eT0_b, (1, 0, 2))
        Db = Dmat.view(np.uint8)                      # [NCHE, 2, 128, CH]
        arena[:, :, :, cfg.D0_OFF:cfg.D0_OFF + cfg.CH] = Db[None, :, 0]
        arena[:, :, :, cfg.D1_OFF:cfg.D1_OFF + cfg.CH] = Db[None, :, 1]
        Ab = Amat.view(np.uint8).reshape(cfg.NCHE, 4, 128, 2 * cfg.AW)
        arena[:, :, :, cfg.A_OFF:cfg.A_OFF + 8 * cfg.AW] = \
            np.transpose(Ab, (0, 2, 1, 3)).reshape(
                cfg.NCHE, 128, 8 * cfg.AW)[None]

        # S_u one-hot [B, EPAD] -> per-chunk u8 rows [NCHE, 64, 2*CH]
        S_u = _onehot(sbat, B)
        suT = np.ascontiguousarray(
            np.transpose(S_u.view(np.uint8).reshape(B, cfg.NCHE, 2 * cfg.CH),
                         (1, 0, 2)))

        # graph-mean one-hot with 1/cnt folded, per node block:
        # [NCHN, 128, NSUB*B] so one DMA covers a node chunk.
        batch_loc = batch[base:base + nl]
        bl_pad = np.concatenate([batch_loc, np.full(nlp - nl, -1, np.int64)])
        Bm = _onehot(bl_pad, B, scale=ginv[np.clip(bl_pad, 0, B - 1)]).T
        Bmat = np.ascontiguousarray(
            Bm.reshape(cfg.NCHN, 4, 128, B).transpose(0, 2, 1, 3).reshape(
                cfg.NCHN, 128, 4 * B))

        xTb0 = np.zeros((128, nlp), BF16)
        xTb0[:, :nl] = x[base:base + nl].T.astype(BF16)

        in_maps.append(dict(
            wpk=wpk, bpk=bpk,
            xTb0=xTb0,
            uT0=np.ascontiguousarray(u.T).astype(np.float32),
            x0b=xb,
            gpair=_wrap16(gidx, cfg.GB),
            arena0=np.ascontiguousarray(arena[0]),
            arena1=np.ascontiguousarray(arena[1]),
            suT=suT,
            Bmat=Bmat,
            S_nb=_onehot(bl_pad, B),
        ))
    return in_maps


# ---------------------------------------------------------------- device program


def build_program(cfg):
    nc = bacc.Bacc("TRN2", target_bir_lowering=False, debug=False,
                   num_devices=cfg.NCORES, num_swdge_queues=4)
    H, B, CH = cfg.H, cfg.B, cfg.CH
    NW = 27
    f32, bf16, i16 = DT.float32, DT.bfloat16, DT.int16
    u8 = DT.uint8

    def din(name, shape, dt):
        return nc.dram_tensor(name, shape, dt, kind="ExternalInput").ap()

    t = {}
    t['wpk'] = din("wpk", [128, NW * 128], bf16)
    t['bpk'] = din("bpk", [128, 15], f32)
    t['xTb0'] = din("xTb0", [128, cfg.NLP], bf16)
    t['uT0'] = din("uT0", [128, B], f32)
    t['x0b'] = din("x0b", [cfg.N, H], bf16)
    t['gpair'] = din("gpair", [128, cfg.EPAD // 16], i16)
    t['arena0'] = din("arena0", [cfg.NCHE, 128, cfg.AR_W], u8)
    t['arena1'] = din("arena1", [cfg.NCHE, 128, cfg.AR_W], u8)
    t['suT'] = din("suT", [cfg.NCHE, B, 2 * CH], u8)
    t['Bmat'] = din("Bmat", [cfg.NCHN, 128, 4 * B], bf16)
    t['S_nb'] = din("S_nb", [B, cfg.NLP], bf16)

    t['out'] = nc.dram_tensor("out", [B, cfg.STEPS, H], f32, kind="ExternalOutput").ap()

    t['x_shard'] = nc.dram_tensor("x_shard", [cfg.NL, H], bf16).ap()
    t['x_full'] = nc.dram_tensor("x_full", [cfg.N, H], bf16, addr_space="Shared").ap()
    t['gsum_in'] = nc.dram_tensor("gsum_in", [128, B], f32).ap()
    t['gsum_out'] = nc.dram_tensor("gsum_out", [128, B], f32, addr_space="Shared").ap()
    t['rg'] = [list(range(cfg.NCORES))]

    with ExitStack() as ctx:
        tc = ctx.enter_context(tile.TileContext(nc))
        _emit(nc, tc, ctx, cfg, t)
    nc.compile()
    return nc


def _emit(nc, tc, ctx, cfg, t):
    H, B, CH = cfg.H, cfg.B, cfg.CH
    f32, bf16, i16 = DT.float32, DT.bfloat16, DT.int16
    u8, f8 = DT.uint8, DT.float8e4
    NSUB = CH // 128

    perm = ctx.enter_context(tc.tile_pool(name="perm", bufs=1))
    sb = ctx.enter_context(tc.tile_pool(name="sb", bufs=3))
    sb2 = ctx.enter_context(tc.tile_pool(name="sb2", bufs=2))
    gp = ctx.enter_context(tc.tile_pool(name="gp", bufs=3))
    ps_h1 = ctx.enter_context(tc.tile_pool(name="ps_h1", bufs=2, space="PSUM"))
    ps_g = ctx.enter_context(tc.tile_pool(name="ps_g", bufs=1, space="PSUM"))
    ps_tp = ctx.enter_context(tc.tile_pool(name="ps_tp", bufs=1, space="PSUM"))

    # ---------------- persistent SBUF state
    W = perm.tile([128, 27 * 128], bf16)
    nc.sync.dma_start(W[:], t['wpk'][:])

    def w(i):
        return W[:, i * 128:(i + 1) * 128]

    bias = perm.tile([128, 15], f32)
    nc.sync.dma_start(bias[:], t['bpk'][:])

    def bv(i):
        return bias[:, i:i + 1]

    xTb = perm.tile([128, cfg.NLP], bf16)
    nc.sync.dma_start(xTb[:], t['xTb0'][:])

    uT = perm.tile([128, B], f32)
    nc.sync.dma_start(uT[:], t['uT0'][:])
    uTb = perm.tile([128, B], bf16)
    nc.vector.tensor_copy(uTb[:], uT[:])

    gpairT = perm.tile([128, cfg.EPAD // 16], i16)
    nc.sync.dma_start(gpairT[:], t['gpair'][:])

    bsum_acc = perm.tile([128, B], f32)
    aggT = perm.tile([128, cfg.NLP], bf16)    # resident aggregation accumulator
    # W1b-projected x rows at two 64-node alignments (for the x[dst] expansion)
    PXa = perm.tile([128, cfg.NBLK, 128], f8)
    PXb = perm.tile([128, cfg.NBLK, 128], f8)

    ident_f = perm.tile([128, 128], f32)
    make_identity(nc, ident_f[:])
    ident_b = perm.tile([128, 128], bf16)
    nc.vector.tensor_copy(ident_b[:], ident_f[:])

    # ---------------- init DRAM state
    nc.sync.dma_start(t['x_full'][:], t['x0b'][:])
    x_lo = t['x_full'][0:SPLIT, :]
    x_hi = t['x_full'][SPLIT:cfg.N, :]

    def gru_tail_acts(pool, pr, pz, pig, phg, bb, FD):
        """Common GRU tail given the four PSUM gate tiles. Returns (n, z)."""
        r = pool.tile([128, FD], bf16, tag="r")
        nc.scalar.activation(r[:], pr[:], AF.Sigmoid, bias=bv(bb + 0))
        z = pool.tile([128, FD], bf16, tag="z")
        nc.scalar.activation(z[:], pz[:], AF.Sigmoid, bias=bv(bb + 1))
        tm = pool.tile([128, FD], bf16, tag="tm")
        if cfg.zero_gbias:
            nc.vector.tensor_tensor(tm[:], r[:], phg[:], op=ALU.mult)
        else:
            hg = pool.tile([128, FD], bf16, tag="hg")
            nc.scalar.activation(hg[:], phg[:], AF.Identity, bias=bv(bb + 2))
            nc.vector.tensor_tensor(tm[:], r[:], hg[:], op=ALU.mult)
        sp = pool.tile([128, FD], f32, tag="sp")
        nc.vector.tensor_tensor(sp[:], tm[:], pig[:], op=ALU.add)
        n = pool.tile([128, FD], bf16, tag="n")
        nc.scalar.activation(n[:], sp[:], AF.Tanh, bias=bv(bb + 3))
        return n, z

    def gru(xiT, hTb, wb, bb, pool, h_f32, out_tag, FD):
        """GRU: xiT bf16 [128,FD] (input through W2 fold), hTb bf16 [128,FD].
        If h_f32 given: blend in f32 in-place there and return None; else return
        a bf16 tile. wb: base index of Wih2 r,z,g then Whh r,z,g. bb: bias base."""
        pr = ps_g.tile([128, FD], f32, tag="pr")
        nc.tensor.matmul(pr[:], lhsT=w(wb + 0), rhs=xiT, start=True, stop=False)
        nc.tensor.matmul(pr[:], lhsT=w(wb + 3), rhs=hTb, start=False, stop=True)
        pz = ps_g.tile([128, FD], f32, tag="pz")
        nc.tensor.matmul(pz[:], lhsT=w(wb + 1), rhs=xiT, start=True, stop=False)
        nc.tensor.matmul(pz[:], lhsT=w(wb + 4), rhs=hTb, start=False, stop=True)
        pig = ps_g.tile([128, FD], f32, tag="pig")
        nc.tensor.matmul(pig[:], lhsT=w(wb + 2), rhs=xiT, start=True, stop=True)
        phg = ps_g.tile([128, FD], f32, tag="phg")
        nc.tensor.matmul(phg[:], lhsT=w(wb + 5), rhs=hTb, start=True, stop=True)

        n, z = gru_tail_acts(pool, pr, pz, pig, phg, bb, FD)
        hold = h_f32 if h_f32 is not None else hTb
        d = pool.tile([128, FD], f32 if h_f32 is not None else bf16, tag="d")
        nc.vector.tensor_tensor(d[:], hold, n[:], op=ALU.subtract)
        m = pool.tile([128, FD], f32 if h_f32 is not None else bf16, tag="m")
        nc.vector.tensor_tensor(m[:], z[:], d[:], op=ALU.mult)
        if h_f32 is not None:
            nc.vector.tensor_tensor(h_f32, n[:], m[:], op=ALU.add)
            return None
        hN = pool.tile([128, FD], bf16, tag=out_tag)
        nc.vector.tensor_tensor(hN[:], n[:], m[:], op=ALU.add)
        return hN

    # SWDGE queue assignment: Tile round-robins DMASW sems (8) over SWDGE
    # instructions in emission order; queue = ctr % num_queues keeps each sem
    # pinned to one queue (sem s -> queue s % 4).
    _swdge_ctr = [0]

    def self_qn(_):
        q = _swdge_ctr[0] % nc.num_swdge_queues
        _swdge_ctr[0] += 1
        return q

    aw_state = [None]

    def emit_agg(k, pend):
        """Transpose chunk k's hN to row form and accumulate the one-hot
        aggregation matmuls. Emitted interleaved into chunk k+1's MM stream."""
        amat_c = pend['amat']
        hN = pend['hN']
        tpp = ps_tp.tile([128, CH], bf16, tag="tp_b")
        erow = sb.tile([128, CH], bf16, tag="erow")
        for j in range(NSUB):
            nc.tensor.transpose(tpp[:, j * 128:(j + 1) * 128],
                                hN[:, j * 128:(j + 1) * 128], ident_b[:])
            if j % 2 == 1:
                nc.vector.tensor_copy(erow[:, (j - 1) * 128:(j + 1) * 128],
                                      tpp[:, (j - 1) * 128:(j + 1) * 128])
        for j in range(NSUB):
            gs = k * NSUB + j
            wb = cfg.wstart[gs]
            first = (gs == 0) or (cfg.grp[gs - 1] != cfg.grp[gs])
            last = (gs == cfg.NSUBS - 1) or (cfg.grp[gs + 1] != cfg.grp[gs])
            if first:
                aw_ps = ps_tp.tile([128, cfg.AW], f32, tag="aw")
                aw_state[0] = aw_ps
            nc.tensor.matmul(aw_state[0][:],
                             lhsT=erow[:, j * 128:(j + 1) * 128],
                             rhs=amat_c[:, j * cfg.AW:(j + 1) * cfg.AW],
                             start=first, stop=last)
            if last:
                nc.vector.tensor_tensor(aggT[:, wb:wb + cfg.AW],
                                        aggT[:, wb:wb + cfg.AW],
                                        aw_state[0][:], op=ALU.add)

    for s in range(cfg.STEPS):
        arena_r = t['arena0'] if s % 2 == 0 else t['arena1']
        arena_w = t['arena1'] if s % 2 == 0 else t['arena0']
        nc.vector.memset(aggT[:], 0.0)

        # per-step u projections: uWd_row = u @ W1d.T ; uWnc_row = u @ Wn1c.T
        uprj = []
        for wi, tg in ((3, "uprj_e"), (12, "uprj_n")):
            p = ps_g.tile([B, 128], f32, tag="pr")
            nc.tensor.matmul(p[:], lhsT=uTb[:], rhs=w(wi), start=True, stop=True)
            srow = sb2.tile([B, 128], bf16, tag=tg)
            nc.vector.tensor_copy(srow[:], p[:])
            uprj.append(srow)
        uWd_row, uWnc_row = uprj

        # PXrow: per 128-node block, rows of x @ W1b.T (two 64-node alignments)
        for dstn, off in ((PXa, 0), (PXb, 64)):
            for blk in range(cfg.NBLK):
                base = off + blk * 128
                wid = min(128, cfg.NLP - base)
                if wid <= 0:
                    break
                px = ps_h1.tile([128, 128], f32, tag="h1")
                nc.tensor.matmul(px[:wid, :], lhsT=xTb[:, base:base + wid],
                                 rhs=w(1), start=True, stop=True)
                nc.vector.tensor_copy(dstn[:, blk, :][:wid, :], px[:wid, :])

        # ================= EDGE PHASE =================
        g_b = None
        pend = None                       # chunk awaiting its aggregation block
        for k in range(cfg.NCHE):
            if k % cfg.CPB == 0:
                b = k // cfg.CPB
                b0 = (k * CH) // 16
                g_b = gp.tile([128, 1, cfg.GB], bf16, tag="g")
                nc.gpsimd.dma_gather(
                    g_b[:], x_lo if b < cfg.NBAT_LO else x_hi,
                    gpairT[:, b0:b0 + cfg.GB // 16], cfg.GB, cfg.GB, H,
                    transpose=True, single_packet=False,
                    queue_num=self_qn(0))

            comb = sb.tile([128, cfg.AR_W], u8, tag="comb")
            nc.sync.dma_start(comb[:], arena_r[k])
            su_t = sb.tile([B, 2 * CH], u8, tag="su")
            nc.sync.dma_start(su_t[:], t['suT'][k])

            eT_c = comb[:, cfg.ET_OFF:cfg.ET_OFF + 2 * CH].bitcast(bf16)
            d0 = comb[:, cfg.D0_OFF:cfg.D0_OFF + CH].bitcast(f8)
            d1 = comb[:, cfg.D1_OFF:cfg.D1_OFF + CH].bitcast(f8)
            amat_c = comb[:, cfg.A_OFF:cfg.A_OFF + 8 * cfg.AW].bitcast(bf16)
            su_c = su_t[:, :].bitcast(bf16)
            g_src = g_b[:, 0, (k % cfg.CPB) * CH:(k % cfg.CPB) * CH + CH]

            w2 = cfg.w2start[k]
            if w2 % 128 == 0:
                pxh0 = PXa[:, w2 // 128, :]
                pxh1 = PXa[:, w2 // 128 + 1, :]
            else:
                pxh0 = PXb[:, (w2 - 64) // 128, :]
                pxh1 = PXb[:, (w2 - 64) // 128 + 1, :]

            # --- PE order: h1 block, gru h-side, prev-chunk agg, gru x-side
            h1 = ps_h1.tile([128, CH], f32, tag="h1")
            nc.tensor.matmul(h1[:], lhsT=w(0), rhs=g_src, start=True, stop=False)
            nc.tensor.matmul(h1[:], lhsT=pxh0, rhs=d0, start=False, stop=False)
            nc.tensor.matmul(h1[:], lhsT=pxh1, rhs=d1, start=False, stop=False)
            nc.tensor.matmul(h1[:], lhsT=w(2), rhs=eT_c, start=False, stop=False)
            nc.tensor.matmul(h1[:], lhsT=uWd_row[:], rhs=su_c, start=False, stop=True)
            rh1 = sb.tile([128, CH], bf16, tag="rh1")
            nc.scalar.activation(rh1[:], h1[:], AF.Relu, bias=bv(0))

            pr = ps_g.tile([128, CH], f32, tag="pr")
            nc.tensor.matmul(pr[:], lhsT=w(7), rhs=eT_c, start=True, stop=False)
            pz = ps_g.tile([128, CH], f32, tag="pz")
            nc.tensor.matmul(pz[:], lhsT=w(8), rhs=eT_c, start=True, stop=False)
            phg = ps_g.tile([128, CH], f32, tag="phg")
            nc.tensor.matmul(phg[:], lhsT=w(9), rhs=eT_c, start=True, stop=True)

            if pend is not None:
                emit_agg(k - 1, pend)

            nc.tensor.matmul(pr[:], lhsT=w(4), rhs=rh1[:], start=False, stop=True)
            nc.tensor.matmul(pz[:], lhsT=w(5), rhs=rh1[:], start=False, stop=True)
            pig = ps_g.tile([128, CH], f32, tag="pig")
            nc.tensor.matmul(pig[:], lhsT=w(6), rhs=rh1[:], start=True, stop=True)

            n, z = gru_tail_acts(sb, pr, pz, pig, phg, 1, CH)
            d = sb.tile([128, CH], bf16, tag="d")
            nc.vector.tensor_tensor(d[:], eT_c, n[:], op=ALU.subtract)
            m = sb.tile([128, CH], bf16, tag="m")
            nc.vector.tensor_tensor(m[:], z[:], d[:], op=ALU.mult)
            hN = sb.tile([128, CH], bf16, tag="hN")
            nc.vector.tensor_tensor(hN[:], n[:], m[:], op=ALU.add)
            if s < cfg.STEPS - 1:
                nc.sync.dma_start(
                    arena_w[k, :, cfg.ET_OFF:cfg.ET_OFF + 2 * CH].bitcast(bf16),
                    hN[:])
            pend = dict(hN=hN, amat=amat_c)
        emit_agg(cfg.NCHE - 1, pend)

        # ================= NODE PHASE =================
        for k in range(cfg.NCHN):
            cn = slice(k * CH, (k + 1) * CH)

            snb_c = sb.tile([B, CH], bf16, tag="snb")
            nc.sync.dma_start(snb_c[:], t['S_nb'][:, cn])

            h1 = ps_h1.tile([128, CH], f32, tag="h1")
            nc.tensor.matmul(h1[:], lhsT=w(10), rhs=xTb[:, cn], start=True, stop=False)
            nc.tensor.matmul(h1[:], lhsT=w(11), rhs=aggT[:, cn], start=False, stop=False)
            nc.tensor.matmul(h1[:], lhsT=uWnc_row[:], rhs=snb_c[:], start=False, stop=True)

            rh1 = sb.tile([128, CH], bf16, tag="rh1")
            nc.scalar.activation(rh1[:], h1[:], AF.Relu, bias=bv(5))

            xN = gru(rh1[:], xTb[:, cn], 13, 6, sb, None, "xN", CH)
            nc.vector.tensor_copy(xTb[:, cn], xN[:])

            # row-form x for AllGather input, graph means
            bmat_c = sb.tile([128, 4 * B], bf16, tag="bmat")
            nc.sync.dma_start(bmat_c[:], t['Bmat'][k])
            bmm = ps_g.tile([128, B], f32, tag="pr")
            for j in range(NSUB):
                blk = k * NSUB + j
                xtp = ps_tp.tile([128, 128], bf16, tag="tp_b")
                nc.tensor.transpose(xtp[:], xTb[:, blk * 128:(blk + 1) * 128],
                                    ident_b[:])
                xrow = sb.tile([128, 128], bf16, tag="xrow")
                nc.vector.tensor_copy(xrow[:], xtp[:])
                base = blk * 128
                nrows = max(0, min(128, cfg.NL - base))
                if nrows > 0 and s < cfg.STEPS - 1:
                    nc.sync.dma_start(t['x_shard'][base:base + nrows, :], xrow[:nrows, :])
                nc.tensor.matmul(bmm[:], lhsT=xrow[:], rhs=bmat_c[:, j * B:(j + 1) * B],
                                 start=(j == 0), stop=(j == NSUB - 1))
            if k == 0:
                nc.vector.tensor_copy(bsum_acc[:], bmm[:])
            else:
                nc.vector.tensor_tensor(bsum_acc[:], bsum_acc[:], bmm[:], op=ALU.add)

        # AllGather x early (before the global phase) so it overlaps it.
        if s < cfg.STEPS - 1:
            nc.gpsimd.collective_compute(
                "AllGather", ALU.bypass, replica_groups=t['rg'],
                ins=[t['x_shard'][:]], outs=[t['x_full'][:]])

        # ================= GLOBAL PHASE =================
        nc.sync.dma_start(t['gsum_in'][:], bsum_acc[:])
        nc.gpsimd.collective_compute(
            "AllReduce", ALU.add, replica_groups=t['rg'],
            ins=[t['gsum_in'][:]], outs=[t['gsum_out'][:]])
        nmF = sb2.tile([128, B], f32, tag="nmF")
        nc.sync.dma_start(nmF[:], t['gsum_out'][:])
        nmT = sb2.tile([128, B], bf16, tag="nmT")
        nc.vector.tensor_copy(nmT[:], nmF[:])

        h1g = ps_h1.tile([128, B], f32, tag="h1")
        nc.tensor.matmul(h1g[:], lhsT=w(19), rhs=uTb[:], start=True, stop=False)
        nc.tensor.matmul(h1g[:], lhsT=w(20), rhs=nmT[:], start=False, stop=True)
        rh1g = sb2.tile([128, B], bf16, tag="rh1g")
        nc.scalar.activation(rh1g[:], h1g[:], AF.Relu, bias=bv(10))

        gru(rh1g[:], uTb[:], 21, 11, sb2, uT[:], None, B)
        nc.vector.tensor_copy(uTb[:], uT[:])

        utp = ps_tp.tile([B, 128], f32, tag="aw")
        nc.tensor.transpose(utp[:], uT[:], ident_f[:])
        urow = sb2.tile([B, 128], f32, tag="urow")
        nc.vector.tensor_copy(urow[:], utp[:])
        nc.sync.dma_start(t['out'][:, s, :], urow[:])


# ---------------------------------------------------------------- entry point

_CACHE = {}


def kernel(**inputs):
    x = np.asarray(inputs['x'])
    ei = np.asarray(inputs['edge_index'])
    u = np.asarray(inputs['u'])
    cfg = Cfg(N=x.shape[0], E=ei.shape[1], B=u.shape[0], H=x.shape[1], STEPS=3)
    in_maps = host_prepare(cfg, inputs)
    key = (cfg.N, cfg.E, cfg.B, cfg.H, cfg.STEPS, cfg.EPAD_LO, cfg.EPAD_HI,
           cfg.zero_gbias)
    if key not in _CACHE:
        _CACHE[key] = build_program(cfg)
    nc = _CACHE[key]
    res = run_bass_kernel_spmd(nc, in_maps, list(range(cfg.NCORES)))
    return np.asarray(res.results[0]["out"], np.float32)
